# revision 1
# baseline (speedup 1.0000x reference)
"""DropStripes (dim=2 SpecAugment) Trainium2 Bass kernel.

x: [64, 1, 4096, 256] f32; bgn, distance: [64, 2] i32.
Zero time stripes [bgn, bgn+distance) along axis 2 per sample.

Sharding: pure data parallel over batch across 8 NeuronCores
(8 samples per core), no communication.

Per-core program (~171us, ~98% of the measured 430GB/s HBM roofline):
  - compute a keep-mask m[128, BL*R] where column b*R+j, partition p
    holds keep(sample b, t = p*R + j); bgn/distance arrive via 0-stride
    broadcast DMAs and the iota table as a constant input (GpSimd
    iota/partition_broadcast would cost ~10us of library swaps)
  - stream x through SBUF in [128, (R/H)*F] quarter-sample tiles:
    partition p holds consecutive t-rows, so every DMA packet is an 8KB
    contiguous run; one tensor_tensor multiply per tile against the
    mask broadcast along the feature axis (free-dim 0-stride), DMA back.
"""
import numpy as np

B, C, T, F = 64, 1, 4096, 256
S = 2
N_CORES = 8
BL = B // N_CORES          # samples per core
R = T // 128               # 32 consecutive t-rows per partition

_cached_nc = None


def _build():
    from contextlib import ExitStack
    import concourse.tile as tile
    from concourse import bacc, mybir

    nc = bacc.Bacc("TRN2", target_bir_lowering=False, debug=False)
    x_d = nc.dram_tensor("x", [BL, T, F], mybir.dt.float32, kind="ExternalInput")
    bgn_d = nc.dram_tensor("bgn", [BL, S], mybir.dt.int32, kind="ExternalInput")
    dist_d = nc.dram_tensor("distance", [BL, S], mybir.dt.int32, kind="ExternalInput")
    tv_d = nc.dram_tensor("tv", [128, R], mybir.dt.int32, kind="ExternalInput")
    out_d = nc.dram_tensor("out", [BL, T, F], mybir.dt.float32, kind="ExternalOutput")

    with tile.TileContext(nc) as tc, ExitStack() as ctx:
        mpool = ctx.enter_context(tc.tile_pool(name="mask", bufs=1))
        xpool = ctx.enter_context(tc.tile_pool(name="x", bufs=12))

        # ---- keep-mask: m[p, b*R + j] = 0 iff t = p*R + j inside a stripe of b
        # No GpSimd ops (iota/partition_broadcast need ~10us library swaps):
        # the t-value table comes in as a constant input, bgn/dist arrive via
        # 0-stride broadcast DMAs, everything else is vector.
        H = 4                    # quarter-sample tiles: 8KB packets
        RH = R // H

        tv = mpool.tile([128, R], mybir.dt.int32)
        nc.scalar.dma_start(tv[:, :], tv_d[:])
        bgn_bc = mpool.tile([128, BL * S], mybir.dt.int32)
        nc.scalar.dma_start(
            bgn_bc[:, :], bgn_d[:].flatten().unsqueeze(0).broadcast_to([128, BL * S]))
        dist_bc = mpool.tile([128, BL * S], mybir.dt.int32)
        nc.scalar.dma_start(
            dist_bc[:, :], dist_d[:].flatten().unsqueeze(0).broadcast_to([128, BL * S]))
        end_bc = mpool.tile([128, BL * S], mybir.dt.int32)
        nc.vector.tensor_add(end_bc[:, :], bgn_bc[:, :], dist_bc[:, :])

        itv = tv[:, :].unsqueeze(1).broadcast_to([128, BL, R])

        def bc(tile_bc, s):
            a = tile_bc[:, s::S]                      # [128, BL]
            return a.unsqueeze(2).broadcast_to([128, BL, R])

        ta = mpool.tile([128, BL * R], mybir.dt.int32)
        tav = ta[:, :].rearrange("p (b j) -> p b j", b=BL)
        tb = mpool.tile([128, BL * R], mybir.dt.int32)
        tbv = tb[:, :].rearrange("p (b j) -> p b j", b=BL)
        acc = mpool.tile([128, BL * R], mybir.dt.int32)
        accv = acc[:, :].rearrange("p (b j) -> p b j", b=BL)

        # stripe 0: acc = (t >= b0) - (t >= e0)   (1 iff inside stripe 0)
        nc.vector.tensor_tensor(tav, itv, bc(bgn_bc, 0), op=mybir.AluOpType.is_ge)
        nc.vector.tensor_tensor(tbv, itv, bc(end_bc, 0), op=mybir.AluOpType.is_ge)
        nc.vector.tensor_sub(accv, tav, tbv)
        # stripe 1, then acc = max(acc, stripe1)
        nc.vector.tensor_tensor(tav, itv, bc(bgn_bc, 1), op=mybir.AluOpType.is_ge)
        nc.vector.tensor_tensor(tbv, itv, bc(end_bc, 1), op=mybir.AluOpType.is_ge)
        nc.vector.tensor_sub(tav, tav, tbv)
        nc.vector.tensor_max(accv, accv, tav)
        # keep = 1 - acc, converted to f32
        m = mpool.tile([128, BL * R], mybir.dt.float32)
        nc.vector.tensor_scalar(
            m[:, :], acc[:, :], -1, 1,
            op0=mybir.AluOpType.mult, op1=mybir.AluOpType.add,
        )

        # ---- stream x: in -> multiply by mask -> out
        # Reads on the sync HW queue, writes on the scalar HW queue; every
        # DMA packet is an 8KB contiguous run, spread over all 16 HW-DGE
        # engines. The mask DMAs above sit at the front of the scalar queue,
        # so the write stream naturally starts ~25us in - which lets the
        # read stream sprint ahead by the full 12-tile window first (fastest
        # measured schedule; eager writes cause a convoy in the tail).
        x_v = x_d[:].rearrange("b (p h j) f -> b h p (j f)", p=128, h=H)
        out_v = out_d[:].rearrange("b (p h j) f -> b h p (j f)", p=128, h=H)
        mv = m[:, :].rearrange("p (b h j) -> p b h j", b=BL, h=H)
        for b in range(BL):
            for h in range(H):
                xt = xpool.tile([128, RH * F], mybir.dt.float32)
                nc.sync.dma_start(xt[:, :], x_v[b, h])
                xtv = xt[:, :].rearrange("p (j f) -> p j f", j=RH)
                mk = mv[:, b, h]
                nc.vector.tensor_tensor(
                    xtv, xtv, mk.unsqueeze(2).broadcast_to([128, RH, F]),
                    op=mybir.AluOpType.mult,
                )
                nc.scalar.dma_start(out_v[b, h], xt[:, :])

    nc.compile()
    return nc


def _in_maps(x, bgn, distance):
    xs = np.ascontiguousarray(x, dtype=np.float32).reshape(B, T, F)
    bgn = np.ascontiguousarray(bgn, dtype=np.int32)
    distance = np.ascontiguousarray(distance, dtype=np.int32)
    # constant t-value table: tv[p, j] = p*R + j
    tv = (np.arange(128, dtype=np.int32)[:, None] * R
          + np.arange(R, dtype=np.int32)[None, :])
    maps = []
    for i in range(N_CORES):
        sl = slice(i * BL, (i + 1) * BL)
        maps.append({
            "x": np.ascontiguousarray(xs[sl]),
            "bgn": np.ascontiguousarray(bgn[sl]),
            "distance": np.ascontiguousarray(distance[sl]),
            "tv": tv,
        })
    return maps


def _get_nc():
    global _cached_nc
    if _cached_nc is None:
        _cached_nc = _build()
    return _cached_nc


def kernel(x, bgn, distance):
    from concourse.bass_utils import run_bass_kernel_spmd

    nc = _get_nc()
    res = run_bass_kernel_spmd(nc, _in_maps(x, bgn, distance),
                               core_ids=list(range(N_CORES)))
    out = np.stack([res.results[i]["out"] for i in range(N_CORES)], axis=0)
    return out.reshape(B, C, T, F)



# revision 2
# speedup vs baseline: 1.6287x; 1.6287x over previous
"""DropStripes (dim=2 SpecAugment) Trainium2 Bass kernel.

x: [64, 1, 4096, 256] f32; bgn, distance: [64, 2] i32.
Zero time stripes [bgn, bgn+distance) along axis 2 per sample.
Pure data parallel over batch across 8 NeuronCores (8 samples/core).

Per-core program (~120us typical; the previous SBUF-bounce+multiply
streaming kernel measured ~202-218us):
  - 8 x 4MiB DRAM->DRAM copies x[b] -> out[b], alternating the two HWDGE
    queues, all issued up front with no waits. No SBUF bounce and no
    vector multiply, so each byte crosses the SDMA engines once instead
    of twice (HBM->SBUF + SBUF->HBM), which is what buys the speedup.
  - per-sample zero fixup: one gpsimd (SWDGE) indirect scatter per
    sample writes a zeros row [256] f32 to up to 126 dynamic row indices
    covering both stripes; padding indices are > bounds_check and
    silently skipped (oob_is_err=False). Row indices are computed
    on-device from bgn/distance with a short int32 vector chain.
  - the dependency tracker treats each scatter's dynamic AP as writing
    all of `out`, which would serialize every copy behind every earlier
    scatter; those conservative deps are pruned down to the only true
    one (scatter b after copy b), letting scatters overlap the stream.
    Cross-sample writes are disjoint (indices stay in sample b's rows).
"""
import numpy as np

B, C, T, F = 64, 1, 4096, 256
S = 2
N_CORES = 8
BL = B // N_CORES          # samples per core
BIG = 1 << 16              # OOB padding row index

_cached_nc = None


def _raw(i):
    return i.ins if hasattr(i, "ins") else i


def _build():
    from contextlib import ExitStack
    import concourse.tile as tile
    from concourse import bacc, bass, mybir

    nc = bacc.Bacc("TRN2", target_bir_lowering=False, debug=False)
    x_d = nc.dram_tensor("x", [BL, T, F], mybir.dt.float32, kind="ExternalInput")
    bgn_d = nc.dram_tensor("bgn", [BL, S], mybir.dt.int32, kind="ExternalInput")
    dist_d = nc.dram_tensor("distance", [BL, S], mybir.dt.int32, kind="ExternalInput")
    # constant table: col 0 = partition index p; col 1+b = b*T
    tab_d = nc.dram_tensor("tab", [128, 1 + BL], mybir.dt.int32, kind="ExternalInput")
    out_d = nc.dram_tensor("out", [BL, T, F], mybir.dt.float32, kind="ExternalOutput")

    with tile.TileContext(nc) as tc, ExitStack() as ctx:
        pool = ctx.enter_context(tc.tile_pool(name="small", bufs=1))

        # --- tiny input DMAs on the gpsimd (SWDGE) queue, keeping both
        # HWDGE queues free for the copy stream.
        tab = pool.tile([128, 1 + BL], mybir.dt.int32)
        nc.gpsimd.dma_start(tab[:, :], tab_d[:])
        bgn_bc = pool.tile([128, BL * S], mybir.dt.int32)
        nc.gpsimd.dma_start(
            bgn_bc[:, :], bgn_d[:].flatten().unsqueeze(0).broadcast_to([128, BL * S]))
        dist_bc = pool.tile([128, BL * S], mybir.dt.int32)
        nc.gpsimd.dma_start(
            dist_bc[:, :], dist_d[:].flatten().unsqueeze(0).broadcast_to([128, BL * S]))

        # --- row-index computation, all [128, BL] int32
        # idx[p, b] = b*T + (bgn0+p          if p < d0
        #                    bgn1 + (p - d0) if p - d0 < d1
        #                    BIG             otherwise)
        P_ = tab[:, 0:1].broadcast_to([128, BL])
        OFF = tab[:, 1:1 + BL]
        b0 = bgn_bc[:, 0::S]
        b1 = bgn_bc[:, 1::S]
        d0 = dist_bc[:, 0::S]
        d1 = dist_bc[:, 1::S]

        def tmp(name):
            return pool.tile([128, BL], mybir.dt.int32, name=name)

        t0 = tmp("t0")
        c0 = tmp("c0")
        pm = tmp("pm")
        c1 = tmp("c1")
        t1 = tmp("t1")
        s1 = tmp("s1")
        w = tmp("w")
        idx = tmp("idx")
        nc.vector.tensor_add(t0[:, :], b0, P_)
        nc.vector.tensor_tensor(c0[:, :], P_, d0, op=mybir.AluOpType.is_lt)
        nc.vector.tensor_sub(pm[:, :], P_, d0)
        nc.vector.tensor_tensor(c1[:, :], pm[:, :], d1, op=mybir.AluOpType.is_lt)
        nc.vector.tensor_add(t1[:, :], b1, pm[:, :])
        # s1 = c1 ? t1 : BIG  ==  c1*(t1-BIG) + BIG
        nc.vector.tensor_scalar(t1[:, :], t1[:, :], -BIG, None, op0=mybir.AluOpType.add)
        nc.vector.tensor_tensor(s1[:, :], c1[:, :], t1[:, :], op=mybir.AluOpType.mult)
        nc.vector.tensor_scalar(s1[:, :], s1[:, :], BIG, None, op0=mybir.AluOpType.add)
        # idx_local = c0 ? t0 : s1  ==  c0*(t0-s1) + s1
        nc.vector.tensor_sub(w[:, :], t0[:, :], s1[:, :])
        nc.vector.tensor_tensor(w[:, :], c0[:, :], w[:, :], op=mybir.AluOpType.mult)
        nc.vector.tensor_add(idx[:, :], w[:, :], s1[:, :])
        nc.vector.tensor_add(idx[:, :], idx[:, :], OFF)

        zrow = pool.tile([128, F], mybir.dt.float32)
        nc.vector.memset(zrow[:, :], 0.0)

        # --- bulk copies (all first, so no copy ever waits on a scatter),
        # then per-sample scatters. The tracker treats each scatter's
        # dynamic AP as writing all of `out`, so it would make scatter b
        # wait for every copy; prune those deps down to the true one
        # (scatter b after copy b) so scatters overlap the copy stream.
        out_flat = out_d[:].rearrange("b t f -> (b t) f")
        cps = []
        for b in range(BL):
            eng = nc.sync if b % 2 == 0 else nc.scalar
            cp = eng.dma_start(
                out_d[b].flatten().unsqueeze(0),
                x_d[b].flatten().unsqueeze(0),
            )
            cps.append(_raw(cp))
        scs = []
        for b in range(BL):
            sc = nc.gpsimd.indirect_dma_start(
                out=out_flat,
                out_offset=bass.IndirectOffsetOnAxis(ap=idx[:, b:b + 1], axis=0),
                in_=zrow[:, :],
                in_offset=None,
                bounds_check=BL * T - 1,
                oob_is_err=False,
            )
            scs.append(_raw(sc))
        cp_names = {c.name for c in cps}
        sc_names = {s.name for s in scs}
        for b, s in enumerate(scs):
            keep = {cps[b].name}
            for n in list(s.sync_dependency_names()):
                if (n in cp_names or n in sc_names) and n not in keep:
                    s.try_remove_dependency(n)
        for c in cps:
            for n in list(c.sync_dependency_names()):
                if n in sc_names:
                    c.try_remove_dependency(n)

    nc.compile()
    return nc


def _in_maps(x, bgn, distance):
    xs = np.ascontiguousarray(x, dtype=np.float32).reshape(B, T, F)
    bgn = np.ascontiguousarray(bgn, dtype=np.int32)
    distance = np.ascontiguousarray(distance, dtype=np.int32)
    tab = np.empty((128, 1 + BL), dtype=np.int32)
    tab[:, 0] = np.arange(128, dtype=np.int32)
    tab[:, 1:] = (np.arange(BL, dtype=np.int32) * T)[None, :]
    maps = []
    for i in range(N_CORES):
        sl = slice(i * BL, (i + 1) * BL)
        maps.append({
            "x": np.ascontiguousarray(xs[sl]),
            "bgn": np.ascontiguousarray(bgn[sl]),
            "distance": np.ascontiguousarray(distance[sl]),
            "tab": tab,
        })
    return maps


def _get_nc():
    global _cached_nc
    if _cached_nc is None:
        _cached_nc = _build()
    return _cached_nc


def kernel(x, bgn, distance):
    from concourse.bass_utils import run_bass_kernel_spmd

    nc = _get_nc()
    res = run_bass_kernel_spmd(nc, _in_maps(x, bgn, distance),
                               core_ids=list(range(N_CORES)))
    out = np.stack([res.results[i]["out"] for i in range(N_CORES)], axis=0)
    return out.reshape(B, C, T, F)
